# revision 40
# baseline (speedup 1.0000x reference)
"""Fused single-head attention (projections + softmax attention) on 8 TRN2
NeuronCores.

Problem: B=4, S=4096, H=1024, D=64
  q = query @ Wq + bq ; k = key @ Wk + bk ; v = value @ Wv + bv
  out = softmax(q k^T / sqrt(D), mask over k) @ v

Sharding: core c -> (batch b = c//2, query half h = c%2). Each core
computes 2048 queries against the full 4096 keys/values of its batch.
No collectives.

Layout strategy (everything chosen so no on-device data ever needs a
slow path):
  - Host feeds bf16 transposed shards qT/kT/vT [H, seq] plus bf16
    weights; biases/mask stay f32.
  - Projections: projT[d, s] = W^T @ xT via W-chunk stationary matmuls;
    psum f32 -> activation(+bias per-partition) -> bf16 SBUF.
  - v additionally PE-transposed tile-wise into v_aug [k, 65] bf16
    with the mask folded in: v_aug = [v*m | m].
  - Scores computed transposed: sT[k, q] = k_projT-tile^T @ q_projT,
    then exp(s/8) -> bf16 expT chunks [128, 32, 512] (one per 512-query
    chunk). No -1e9 masking and no max-subtraction needed: |s| <~ 4,
    and masking rides in v_aug.
  - att@v swapped: outT[*, q] += v_aug[t]^T @ expT[t] — stationary
    v_aug, N=512 moving. Row 64 is the softmax denominator.
    out = num * broadcast(1/den) via DVE reciprocal + rank-1 ones
    matmul + one vector multiply; written transposed [64, 2048] and
    un-transposed on the host.

Perf notes (measured on TRN2, whole-NEFF exec ~160 us/core):
  - All matmuls padded to K=128 / M=128 (zero rows/cols): the PE HAM
    clock gate only promotes 1.2 -> 2.4 GHz when the array looks busy;
    half-array matmuls keep it throttled.
  - Inputs stream as 3-way split DMAs over the gpsimd SWDGE + scalar
    HWDGE + sync HWDGE queues (one queue alone is ~80-125 GB/s).
  - Program emission is sorted by data arrival (k -> q -> v) with v
    projection/transposes and attv t-ranges woven between score chunks
    so the PE (in-order) and the Scalar exp stream both stay fed.
  - exp runs on [128, 1024] PSUM APs, 2 key tiles per activation.
"""

import ml_dtypes
import numpy as np

import concourse.bass as bass
import concourse.mybir as mybir
import concourse.tile as tile
from concourse.masks import make_identity
from concourse.vector_clock import ScopedClock

B, S, H, D = 4, 4096, 1024, 64
NCORES = 8
SQ = S // 2          # queries per core
HT = H // 128        # 8 contraction chunks
TK = S // 128        # 32 key tiles
QCH = 512            # matmul moving free dim
NQC = SQ // QCH      # 4 query chunks per core

FP = mybir.dt.float32
BF = mybir.dt.bfloat16

# ---------------------------------------------------------------------------
# Walrus in this container rejects >1 sync-wait per instruction; peel extra
# waits onto same-engine nops (engine streams are in-order).
_orig_commit = tile.TileContext._commit_instruction


def _split_waits(self, inst):
    si = inst.sync_info
    if si is None or not si.on_wait or len(si.on_wait) <= 1:
        return
    waits = list(si.on_wait)
    si.on_wait = waits[-1:]
    for w in waits[:-1]:
        nop = mybir.InstNoOp(
            name=self.nc.get_next_instruction_name(),
            sync_info=mybir.SyncInfo(on_wait=[w], on_update=[]),
            bass_nofuse=True,
            engine=inst.engine,
            ins=[],
            outs=[],
        )
        _orig_commit(self, nop)


def _patched_commit(self, inst, lazy_reg_writes=True):
    _split_waits(self, inst)
    return _orig_commit(self, inst, lazy_reg_writes)


def _patched_drain_and_barrier(self, tick_clock, wait_clock):
    nc = self.nc
    collector = nc.sync.nop(nofuse=True, hint="tile_drain_waits")
    wait_clock.add_sem_waits(
        collector.ins, ScopedClock({None: tick_clock.global_clock})
    )
    si = collector.ins.sync_info
    if si is not None and si.on_wait and len(si.on_wait) > 1:
        waits = list(si.on_wait)
        si.on_wait = waits[:1]
        for w in waits[1:]:
            extra = nc.sync.nop(nofuse=True, hint="tile_drain_waits")
            if extra.ins.sync_info is None:
                extra.ins.sync_info = mybir.SyncInfo(on_wait=[w], on_update=[])
            else:
                extra.ins.sync_info.on_wait = [w]
    nc.sync.drain()
    nc.all_engine_barrier()
    assert self.sems is not None
    popped = nc._tile_sem_poison_stack.pop()
    assert popped is self._sem_poison
    nc.clear_and_free_semaphores(list(self.sems.allocated().values()))
    nc.all_engine_barrier()


tile.TileContext._commit_instruction = _patched_commit
tile.TileContext._drain_and_barrier = _patched_drain_and_barrier
# ---------------------------------------------------------------------------

AF = mybir.ActivationFunctionType


def _build():
    nc = bass.Bass(trn_type="TRN2")

    qT = nc.declare_dram_parameter("qT", [H, SQ], BF, isOutput=False)
    kT = nc.declare_dram_parameter("kT", [H, S], BF, isOutput=False)
    vT = nc.declare_dram_parameter("vT", [H, S], BF, isOutput=False)
    maskT = nc.declare_dram_parameter("maskT", [128, TK], FP, isOutput=False)
    wq = nc.declare_dram_parameter("wq", [H, D], BF, isOutput=False)
    wk = nc.declare_dram_parameter("wk", [H, D], BF, isOutput=False)
    wv = nc.declare_dram_parameter("wv", [H, D], BF, isOutput=False)
    bq = nc.declare_dram_parameter("bq", [D, 1], FP, isOutput=False)
    bk = nc.declare_dram_parameter("bk", [D, 1], FP, isOutput=False)
    bv = nc.declare_dram_parameter("bv", [D, 1], FP, isOutput=False)
    outT = nc.declare_dram_parameter("outT", [D, SQ], FP, isOutput=True)

    qT_ap = qT[:, :].rearrange("(o p) s -> p o s", p=128)
    kT_ap = kT[:, :].rearrange("(o p) s -> p o s", p=128)
    vT_ap = vT[:, :].rearrange("(o p) s -> p o s", p=128)
    wq_ap = wq[:, :].rearrange("(o p) d -> p o d", p=128)
    wk_ap = wk[:, :].rearrange("(o p) d -> p o d", p=128)
    wv_ap = wv[:, :].rearrange("(o p) d -> p o d", p=128)

    with tile.TileContext(nc) as tc:
        with (
            tc.tile_pool(name="const", bufs=1) as cpool,
            tc.tile_pool(name="proj", bufs=1) as projpool,
            tc.tile_pool(name="xin", bufs=3) as xpool,
            tc.tile_pool(name="expb", bufs=3) as exppool,
            tc.tile_pool(name="outs", bufs=1) as outpool,
            tc.tile_pool(name="big", bufs=3, space="PSUM") as ps_big,
            tc.tile_pool(name="att", bufs=2, space="PSUM") as ps_att,
        ):
            # ---- constants ------------------------------------------------
            wq_s = cpool.tile([128, HT, D], BF, tag="wq")
            wk_s = cpool.tile([128, HT, D], BF, tag="wk")
            wv_s = cpool.tile([128, HT, D], BF, tag="wv")
            nc.scalar.dma_start(wk_s[:], wk_ap)
            nc.sync.dma_start(wq_s[:], wq_ap)
            nc.sync.dma_start(wv_s[:], wv_ap)
            bq_s = cpool.tile([D, 1], FP, tag="bq")
            bk_s = cpool.tile([D, 1], FP, tag="bk")
            bv_s = cpool.tile([D, 1], FP, tag="bv")
            nc.sync.dma_start(bq_s[:], bq[:, :])
            nc.sync.dma_start(bk_s[:], bk[:, :])
            nc.sync.dma_start(bv_s[:], bv[:, :])
            maskT_s = cpool.tile([128, TK], FP, tag="mask")
            nc.sync.dma_start(maskT_s[:], maskT[:, :])
            ones64 = cpool.tile([1, D], FP, tag="ones")
            nc.vector.memset(ones64[:], 1.0)

            # ---- projections: {q,k,v}_projT [64, seq] bf16 ----------------
            q_projT = projpool.tile([128, SQ], BF, tag="qproj")
            k_projT = projpool.tile([128, S], BF, tag="kproj")
            v_projT = projpool.tile([D, S], BF, tag="vproj")
            nc.vector.memset(q_projT[D:, :], 0.0)
            nc.vector.memset(k_projT[D:, :], 0.0)

            CW = 2 * QCH  # default 1024-col input chunks

            def proj_chunk(nm, dst, src_ap, w_s, b_s, c0, cw):
                xt = xpool.tile(
                    [128, HT, CW], BF, tag="xin", name=f"x{nm}{c0}"
                )
                c1 = c0 + cw
                nc.gpsimd.dma_start(xt[:, 0:3, :cw], src_ap[:, 0:3, c0:c1])
                nc.scalar.dma_start(xt[:, 3:6, :cw], src_ap[:, 3:6, c0:c1])
                nc.sync.dma_start(xt[:, 6:8, :cw], src_ap[:, 6:8, c0:c1])
                ps = ps_big.tile([128, CW], FP, tag="big", name=f"ps{nm}{c0}")
                for j in range(cw // QCH):
                    for o in range(HT):
                        nc.tensor.matmul(
                            ps[:D, j * QCH : (j + 1) * QCH],
                            w_s[:, o, :],
                            xt[:, o, j * QCH : (j + 1) * QCH],
                            start=(o == 0),
                            stop=(o == HT - 1),
                        )
                nc.vector.tensor_scalar_add(
                    dst[:D, c0:c1], ps[:D, :cw], b_s[:, :]
                )

            def k_chunk(c0, cw):
                proj_chunk("k", k_projT, kT_ap, wk_s, bk_s, c0, cw)

            def q_chunk(c0, cw=CW):
                proj_chunk("q", q_projT, qT_ap, wq_s, bq_s, c0, cw)

            ident = cpool.tile([D, D], BF, tag="ident")
            make_identity(nc, ident[:])

            # ---- v_aug [128, TK, 128] bf16 = [v*m | m | 0] -----------------
            # v chunks + transposes are emitted as fillers inside the first
            # two scores chunks so PE always has runnable work while vT
            # streams in.
            v_aug = projpool.tile([128, TK, 128], BF, tag="vaug")
            nc.vector.memset(v_aug[:, :, D + 1 :], 0.0)

            def v_chunk(j):
                proj_chunk("v", v_projT, vT_ap, wv_s, bv_s, j * CW, CW)

            def v_trans(j):
                for t in range(8 * j, 8 * j + 8):
                    tp = ps_big.tile(
                        [128, 2 * QCH], BF, tag="big", name=f"tp{t}"
                    )
                    nc.tensor.transpose(
                        tp[:, :D],
                        v_projT[:, t * 128 : (t + 1) * 128],
                        ident[:, :],
                    )
                    nc.vector.tensor_scalar_mul(
                        v_aug[:, t, :D], tp[:, :D], maskT_s[:, t : t + 1]
                    )
                    nc.vector.tensor_copy(
                        v_aug[:, t, D : D + 1], maskT_s[:, t : t + 1]
                    )

            # ---- attention, software-pipelined over query chunks ----------
            # scores(c): 32 matmuls [128,512] + 16 exp [128,1024] -> expT(c)
            # attv(c):   32 matmuls accumulating [65,512] over key tiles
            # program order: s(0) s(1) a(0) s(2) a(1) s(3) a(2) a(3)
            outT_s = outpool.tile([D, SQ], FP, tag="outT")
            exp_tiles = {}

            def scores_part(c, lo, hi):
                if c not in exp_tiles:
                    exp_tiles[c] = exppool.tile(
                        [128, TK, QCH], BF, tag="expT", name=f"expT{c}"
                    )
                expTc = exp_tiles[c]
                q0 = c * QCH
                for tp_ in range(lo, hi, 2):
                    sp = ps_big.tile([128, 2 * QCH], FP, tag="big", name=f"sp{c}_{tp_}")
                    for j in range(2):
                        t = tp_ + j
                        nc.tensor.matmul(
                            sp[:, j * QCH : (j + 1) * QCH],
                            k_projT[:, t * 128 : (t + 1) * 128],
                            q_projT[:, q0 : q0 + QCH],
                            start=True,
                            stop=True,
                        )
                    nc.scalar.activation(
                        expTc[:, tp_ : tp_ + 2, :],
                        sp[:],
                        AF.Exp,
                        scale=0.125,
                    )

            att_ps = {}

            def attv_part(c, lo, hi):
                if c not in att_ps:
                    att_ps[c] = ps_att.tile(
                        [128, QCH], FP, tag="att", name=f"att{c}"
                    )
                ap = att_ps[c]
                expTc = exp_tiles[c]
                for t in range(lo, hi):
                    nc.tensor.matmul(
                        ap[:, :],
                        v_aug[:, t, :],
                        expTc[:, t, :],
                        start=(t == 0),
                        stop=(t == TK - 1),
                    )

            def attv_fin(c):
                ap = att_ps.pop(c)
                exp_tiles.pop(c)
                recip = outpool.tile([1, QCH], FP, tag="recip", name=f"recip{c}")
                nc.vector.reciprocal(recip[:], ap[D : D + 1, :])
                rb = ps_big.tile([128, 2 * QCH], FP, tag="big", name=f"rb{c}")
                nc.tensor.matmul(
                    rb[:D, :QCH], ones64[:, :], recip[:, :], start=True, stop=True
                )
                rbs = outpool.tile([D, QCH], FP, tag="rbs", name=f"rbs{c}")
                nc.vector.tensor_copy(rbs[:], rb[:D, :QCH])
                nc.vector.tensor_tensor(
                    outT_s[:, c * QCH : (c + 1) * QCH],
                    ap[:D, :],
                    rbs[:],
                    mybir.AluOpType.mult,
                )
                eng = nc.gpsimd if c % 2 == 0 else nc.scalar
                eng.dma_start(
                    outT[:, c * QCH : (c + 1) * QCH],
                    outT_s[:, c * QCH : (c + 1) * QCH],
                )

            # Emission order sorted by data arrival. Stream order on the DMA
            # queues: k0 q0 k1 k2 k3 k4 q1 v0 v1 v2 v3. Scores for chunks
            # 0/1 (q cols 0:1024) start as soon as the first k tiles land;
            # v work and attv t-ranges chase the v stream.
            k_chunk(0, 512)      # k tiles 0-3
            q_chunk(0, 512)      # q cols 0:512 (score chunk 0)
            scores_part(0, 0, 4)
            k_chunk(512, 512)    # k tiles 4-7
            q_chunk(512, 512)    # q cols 512:1024 (score chunk 1)
            scores_part(0, 4, 8)
            k_chunk(1024, 1024)  # k tiles 8-15
            scores_part(0, 8, 16)
            k_chunk(2048, 1024)  # k tiles 16-23
            scores_part(0, 16, 24)
            k_chunk(3072, 1024)  # k tiles 24-31
            scores_part(0, 24, 32)
            v_chunk(0)
            scores_part(1, 0, 16)
            q_chunk(CW)          # q cols 1024:2048 (score chunks 2 and 3)
            v_trans(0)
            scores_part(1, 16, 32)
            v_chunk(1)
            v_trans(1)
            attv_part(0, 0, 16)
            v_chunk(2)
            v_trans(2)
            scores_part(2, 0, 8)
            attv_part(0, 16, 24)
            v_chunk(3)
            v_trans(3)
            scores_part(2, 8, 16)
            attv_part(0, 24, 32)
            attv_fin(0)
            scores_part(2, 16, 32)
            attv_part(1, 0, 32)
            attv_fin(1)
            scores_part(3, 0, 32)
            attv_part(2, 0, 32)
            attv_fin(2)
            attv_part(3, 0, 32)
            attv_fin(3)


    return nc


_NC_CACHE = None
LAST_RESULT = None


def kernel(query, key, value, mask, Wq, bq, Wk, bk, Wv, bv):
    global _NC_CACHE, LAST_RESULT
    from concourse.bass_utils import run_bass_kernel_spmd

    bf16 = ml_dtypes.bfloat16
    query = np.asarray(query, np.float32)
    key = np.asarray(key, np.float32)
    value = np.asarray(value, np.float32)
    maskf = np.asarray(mask).astype(np.float32)
    Wqb = np.asarray(Wq, np.float32).astype(bf16)
    Wkb = np.asarray(Wk, np.float32).astype(bf16)
    Wvb = np.asarray(Wv, np.float32).astype(bf16)
    bq = np.asarray(bq, np.float32).reshape(D, 1)
    bk = np.asarray(bk, np.float32).reshape(D, 1)
    bv = np.asarray(bv, np.float32).reshape(D, 1)

    in_maps = []
    for c in range(NCORES):
        b, h = divmod(c, 2)
        qs = slice(h * SQ, (h + 1) * SQ)
        in_maps.append(
            {
                "qT": np.ascontiguousarray(query[b, qs].T).astype(bf16),
                "kT": np.ascontiguousarray(key[b].T).astype(bf16),
                "vT": np.ascontiguousarray(value[b].T).astype(bf16),
                "maskT": np.ascontiguousarray(maskf[b].reshape(TK, 128).T),
                "wq": Wqb,
                "wk": Wkb,
                "wv": Wvb,
                "bq": bq,
                "bk": bk,
                "bv": bv,
            }
        )

    if _NC_CACHE is None:
        _NC_CACHE = _build()

    res = run_bass_kernel_spmd(
        _NC_CACHE, in_maps, core_ids=list(range(NCORES))
    )
    LAST_RESULT = res

    outv = np.empty((B, S, D), np.float32)
    for c in range(NCORES):
        b, h = divmod(c, 2)
        outv[b, h * SQ : (h + 1) * SQ] = res.results[c]["outT"].T
    return outv


# revision 41
# speedup vs baseline: 1.0015x; 1.0015x over previous
"""Fused single-head attention (projections + softmax attention) on 8 TRN2
NeuronCores.

Problem: B=4, S=4096, H=1024, D=64
  q = query @ Wq + bq ; k = key @ Wk + bk ; v = value @ Wv + bv
  out = softmax(q k^T / sqrt(D), mask over k) @ v

Sharding: core c -> (batch b = c//2, query half h = c%2). Each core
computes 2048 queries against the full 4096 keys/values of its batch.
No collectives.

Layout strategy (everything chosen so no on-device data ever needs a
slow path):
  - Host feeds bf16 transposed shards qT/kT/vT [H, seq] plus bf16
    weights; biases/mask stay f32.
  - Projections: projT[d, s] = W^T @ xT via W-chunk stationary matmuls;
    psum f32 -> activation(+bias per-partition) -> bf16 SBUF.
  - v additionally PE-transposed tile-wise into v_aug [k, 65] bf16
    with the mask folded in: v_aug = [v*m | m].
  - Scores computed transposed: sT[k, q] = k_projT-tile^T @ q_projT,
    then exp(s/8) -> bf16 expT chunks [128, 32, 512] (one per 512-query
    chunk). No -1e9 masking and no max-subtraction needed: |s| <~ 4,
    and masking rides in v_aug.
  - att@v swapped: outT[*, q] += v_aug[t]^T @ expT[t] — stationary
    v_aug, N=512 moving. Row 64 is the softmax denominator.
    out = num * broadcast(1/den) via DVE reciprocal + rank-1 ones
    matmul + one vector multiply; written transposed [64, 2048] and
    un-transposed on the host.

Perf notes (measured on TRN2, whole-NEFF exec ~160 us/core):
  - All matmuls padded to K=128 / M=128 (zero rows/cols): the PE HAM
    clock gate only promotes 1.2 -> 2.4 GHz when the array looks busy;
    half-array matmuls keep it throttled.
  - Inputs stream as 3-way split DMAs over the gpsimd SWDGE + scalar
    HWDGE + sync HWDGE queues (one queue alone is ~80-125 GB/s).
  - Program emission is sorted by data arrival (k -> q -> v) with v
    projection/transposes and attv t-ranges woven between score chunks
    so the PE (in-order) and the Scalar exp stream both stay fed.
  - exp runs on [128, 1024] PSUM APs, 2 key tiles per activation.
"""

import ml_dtypes
import numpy as np

import concourse.bass as bass
import concourse.mybir as mybir
import concourse.tile as tile
from concourse.masks import make_identity
from concourse.vector_clock import ScopedClock

B, S, H, D = 4, 4096, 1024, 64
NCORES = 8
SQ = S // 2          # queries per core
HT = H // 128        # 8 contraction chunks
TK = S // 128        # 32 key tiles
QCH = 512            # matmul moving free dim
NQC = SQ // QCH      # 4 query chunks per core

FP = mybir.dt.float32
BF = mybir.dt.bfloat16

# ---------------------------------------------------------------------------
# Walrus in this container rejects >1 sync-wait per instruction; peel extra
# waits onto same-engine nops (engine streams are in-order).
_orig_commit = tile.TileContext._commit_instruction


def _split_waits(self, inst):
    si = inst.sync_info
    if si is None or not si.on_wait or len(si.on_wait) <= 1:
        return
    waits = list(si.on_wait)
    si.on_wait = waits[-1:]
    for w in waits[:-1]:
        nop = mybir.InstNoOp(
            name=self.nc.get_next_instruction_name(),
            sync_info=mybir.SyncInfo(on_wait=[w], on_update=[]),
            bass_nofuse=True,
            engine=inst.engine,
            ins=[],
            outs=[],
        )
        _orig_commit(self, nop)


def _patched_commit(self, inst, lazy_reg_writes=True):
    _split_waits(self, inst)
    return _orig_commit(self, inst, lazy_reg_writes)


def _patched_drain_and_barrier(self, tick_clock, wait_clock):
    nc = self.nc
    collector = nc.sync.nop(nofuse=True, hint="tile_drain_waits")
    wait_clock.add_sem_waits(
        collector.ins, ScopedClock({None: tick_clock.global_clock})
    )
    si = collector.ins.sync_info
    if si is not None and si.on_wait and len(si.on_wait) > 1:
        waits = list(si.on_wait)
        si.on_wait = waits[:1]
        for w in waits[1:]:
            extra = nc.sync.nop(nofuse=True, hint="tile_drain_waits")
            if extra.ins.sync_info is None:
                extra.ins.sync_info = mybir.SyncInfo(on_wait=[w], on_update=[])
            else:
                extra.ins.sync_info.on_wait = [w]
    nc.sync.drain()
    nc.all_engine_barrier()
    assert self.sems is not None
    popped = nc._tile_sem_poison_stack.pop()
    assert popped is self._sem_poison
    nc.clear_and_free_semaphores(list(self.sems.allocated().values()))
    nc.all_engine_barrier()


tile.TileContext._commit_instruction = _patched_commit
tile.TileContext._drain_and_barrier = _patched_drain_and_barrier
# ---------------------------------------------------------------------------

AF = mybir.ActivationFunctionType


def _build():
    nc = bass.Bass(trn_type="TRN2")

    qT = nc.declare_dram_parameter("qT", [H, SQ], BF, isOutput=False)
    kT = nc.declare_dram_parameter("kT", [H, S], BF, isOutput=False)
    vT = nc.declare_dram_parameter("vT", [H, S], BF, isOutput=False)
    maskT = nc.declare_dram_parameter("maskT", [128, TK], FP, isOutput=False)
    wq = nc.declare_dram_parameter("wq", [H, D], BF, isOutput=False)
    wk = nc.declare_dram_parameter("wk", [H, D], BF, isOutput=False)
    wv = nc.declare_dram_parameter("wv", [H, D], BF, isOutput=False)
    bq = nc.declare_dram_parameter("bq", [D, 1], FP, isOutput=False)
    bk = nc.declare_dram_parameter("bk", [D, 1], FP, isOutput=False)
    bv = nc.declare_dram_parameter("bv", [D, 1], FP, isOutput=False)
    outT = nc.declare_dram_parameter("outT", [D, SQ], FP, isOutput=True)

    qT_ap = qT[:, :].rearrange("(o p) s -> p o s", p=128)
    kT_ap = kT[:, :].rearrange("(o p) s -> p o s", p=128)
    vT_ap = vT[:, :].rearrange("(o p) s -> p o s", p=128)
    wq_ap = wq[:, :].rearrange("(o p) d -> p o d", p=128)
    wk_ap = wk[:, :].rearrange("(o p) d -> p o d", p=128)
    wv_ap = wv[:, :].rearrange("(o p) d -> p o d", p=128)

    with tile.TileContext(nc) as tc:
        with (
            tc.tile_pool(name="const", bufs=1) as cpool,
            tc.tile_pool(name="proj", bufs=1) as projpool,
            tc.tile_pool(name="xin", bufs=3) as xpool,
            tc.tile_pool(name="expb", bufs=3) as exppool,
            tc.tile_pool(name="outs", bufs=1) as outpool,
            tc.tile_pool(name="big", bufs=3, space="PSUM") as ps_big,
            tc.tile_pool(name="att", bufs=2, space="PSUM") as ps_att,
        ):
            # ---- constants ------------------------------------------------
            wq_s = cpool.tile([128, HT, D], BF, tag="wq")
            wk_s = cpool.tile([128, HT, D], BF, tag="wk")
            wv_s = cpool.tile([128, HT, D], BF, tag="wv")
            nc.scalar.dma_start(wk_s[:], wk_ap)
            nc.sync.dma_start(wq_s[:], wq_ap)
            nc.sync.dma_start(wv_s[:], wv_ap)
            bq_s = cpool.tile([D, 1], FP, tag="bq")
            bk_s = cpool.tile([D, 1], FP, tag="bk")
            bv_s = cpool.tile([D, 1], FP, tag="bv")
            nc.sync.dma_start(bq_s[:], bq[:, :])
            nc.sync.dma_start(bk_s[:], bk[:, :])
            nc.sync.dma_start(bv_s[:], bv[:, :])
            maskT_s = cpool.tile([128, TK], FP, tag="mask")
            nc.sync.dma_start(maskT_s[:], maskT[:, :])
            ones64 = cpool.tile([1, D], FP, tag="ones")
            nc.vector.memset(ones64[:], 1.0)

            # ---- projections: {q,k,v}_projT [64, seq] bf16 ----------------
            q_projT = projpool.tile([128, SQ], BF, tag="qproj")
            k_projT = projpool.tile([128, S], BF, tag="kproj")
            v_projT = projpool.tile([D, S], BF, tag="vproj")
            nc.vector.memset(q_projT[D:, :], 0.0)
            nc.vector.memset(k_projT[D:, :], 0.0)

            CW = 2 * QCH  # default 1024-col input chunks

            def proj_chunk(nm, dst, src_ap, w_s, b_s, c0, cw):
                xt = xpool.tile(
                    [128, HT, CW], BF, tag="xin", name=f"x{nm}{c0}"
                )
                c1 = c0 + cw
                nc.gpsimd.dma_start(xt[:, 0:3, :cw], src_ap[:, 0:3, c0:c1])
                nc.scalar.dma_start(xt[:, 3:6, :cw], src_ap[:, 3:6, c0:c1])
                nc.sync.dma_start(xt[:, 6:8, :cw], src_ap[:, 6:8, c0:c1])
                ps = ps_big.tile([128, CW], FP, tag="big", name=f"ps{nm}{c0}")
                for j in range(cw // QCH):
                    for o in range(HT):
                        nc.tensor.matmul(
                            ps[:D, j * QCH : (j + 1) * QCH],
                            w_s[:, o, :],
                            xt[:, o, j * QCH : (j + 1) * QCH],
                            start=(o == 0),
                            stop=(o == HT - 1),
                        )
                nc.vector.tensor_scalar_add(
                    dst[:D, c0:c1], ps[:D, :cw], b_s[:, :]
                )

            def k_chunk(c0, cw):
                proj_chunk("k", k_projT, kT_ap, wk_s, bk_s, c0, cw)

            def q_chunk(c0, cw=CW):
                proj_chunk("q", q_projT, qT_ap, wq_s, bq_s, c0, cw)

            ident = cpool.tile([D, D], BF, tag="ident")
            make_identity(nc, ident[:])

            # ---- v_aug [128, TK, 128] bf16 = [v*m | m | 0] -----------------
            # v chunks + transposes are emitted as fillers inside the first
            # two scores chunks so PE always has runnable work while vT
            # streams in.
            v_aug = projpool.tile([128, TK, 128], BF, tag="vaug")
            nc.vector.memset(v_aug[:, :, D + 1 :], 0.0)

            def v_chunk(j):
                proj_chunk("v", v_projT, vT_ap, wv_s, bv_s, j * CW, CW)

            def v_trans(j):
                for t in range(8 * j, 8 * j + 8):
                    tp = ps_big.tile(
                        [128, 2 * QCH], BF, tag="big", name=f"tp{t}"
                    )
                    nc.tensor.transpose(
                        tp[:, :D],
                        v_projT[:, t * 128 : (t + 1) * 128],
                        ident[:, :],
                    )
                    nc.vector.tensor_scalar_mul(
                        v_aug[:, t, :D], tp[:, :D], maskT_s[:, t : t + 1]
                    )
                    nc.vector.tensor_copy(
                        v_aug[:, t, D : D + 1], maskT_s[:, t : t + 1]
                    )

            # ---- attention, software-pipelined over query chunks ----------
            # scores(c): 32 matmuls [128,512] + 16 exp [128,1024] -> expT(c)
            # attv(c):   32 matmuls accumulating [65,512] over key tiles
            # program order: s(0) s(1) a(0) s(2) a(1) s(3) a(2) a(3)
            outT_s = outpool.tile([D, SQ], FP, tag="outT")
            exp_tiles = {}

            def scores_part(c, lo, hi):
                if c not in exp_tiles:
                    exp_tiles[c] = exppool.tile(
                        [128, TK, QCH], BF, tag="expT", name=f"expT{c}"
                    )
                expTc = exp_tiles[c]
                q0 = c * QCH
                for tp_ in range(lo, hi, 2):
                    sp = ps_big.tile([128, 2 * QCH], FP, tag="big", name=f"sp{c}_{tp_}")
                    for j in range(2):
                        t = tp_ + j
                        nc.tensor.matmul(
                            sp[:, j * QCH : (j + 1) * QCH],
                            k_projT[:, t * 128 : (t + 1) * 128],
                            q_projT[:, q0 : q0 + QCH],
                            start=True,
                            stop=True,
                        )
                    nc.scalar.activation(
                        expTc[:, tp_ : tp_ + 2, :],
                        sp[:],
                        AF.Exp,
                        scale=0.125,
                    )

            att_ps = {}

            def attv_part(c, lo, hi):
                if c not in att_ps:
                    att_ps[c] = ps_att.tile(
                        [128, QCH], FP, tag="att", name=f"att{c}"
                    )
                ap = att_ps[c]
                expTc = exp_tiles[c]
                for t in range(lo, hi):
                    nc.tensor.matmul(
                        ap[:, :],
                        v_aug[:, t, :],
                        expTc[:, t, :],
                        start=(t == 0),
                        stop=(t == TK - 1),
                    )

            def attv_fin(c):
                ap = att_ps.pop(c)
                exp_tiles.pop(c)
                recip = outpool.tile([1, QCH], FP, tag="recip", name=f"recip{c}")
                nc.vector.reciprocal(recip[:], ap[D : D + 1, :])
                rb = ps_big.tile([128, 2 * QCH], FP, tag="big", name=f"rb{c}")
                nc.tensor.matmul(
                    rb[:D, :QCH], ones64[:, :], recip[:, :], start=True, stop=True
                )
                rbs = outpool.tile([D, QCH], FP, tag="rbs", name=f"rbs{c}")
                nc.vector.tensor_copy(rbs[:], rb[:D, :QCH])
                nc.vector.tensor_tensor(
                    outT_s[:, c * QCH : (c + 1) * QCH],
                    ap[:D, :],
                    rbs[:],
                    mybir.AluOpType.mult,
                )
                eng = nc.gpsimd if c % 2 == 0 else nc.scalar
                eng.dma_start(
                    outT[:, c * QCH : (c + 1) * QCH],
                    outT_s[:, c * QCH : (c + 1) * QCH],
                )

            # Emission order sorted by data arrival. Stream order on the DMA
            # queues: k0 q0 k1 k2 k3 k4 q1 v0 v1 v2 v3. Scores for chunks
            # 0/1 (q cols 0:1024) start as soon as the first k tiles land;
            # v work and attv t-ranges chase the v stream.
            k_chunk(0, 512)      # k tiles 0-3
            q_chunk(0, 512)      # q cols 0:512 (score chunk 0)
            scores_part(0, 0, 4)
            k_chunk(512, 512)    # k tiles 4-7
            q_chunk(512, 512)    # q cols 512:1024 (score chunk 1)
            scores_part(0, 4, 8)
            k_chunk(1024, 1024)  # k tiles 8-15
            scores_part(0, 8, 16)
            k_chunk(2048, 1024)  # k tiles 16-23
            scores_part(0, 16, 24)
            k_chunk(3072, 1024)  # k tiles 24-31
            scores_part(0, 24, 32)
            q_chunk(CW)          # q cols 1024:2048 (score chunks 2 and 3)
            scores_part(1, 0, 16)
            v_chunk(0)
            v_trans(0)
            scores_part(1, 16, 32)
            v_chunk(1)
            v_trans(1)
            attv_part(0, 0, 16)
            v_chunk(2)
            v_trans(2)
            scores_part(2, 0, 8)
            attv_part(0, 16, 24)
            v_chunk(3)
            v_trans(3)
            scores_part(2, 8, 16)
            attv_part(0, 24, 32)
            attv_fin(0)
            scores_part(2, 16, 32)
            attv_part(1, 0, 32)
            attv_fin(1)
            scores_part(3, 0, 32)
            attv_part(2, 0, 32)
            attv_fin(2)
            attv_part(3, 0, 32)
            attv_fin(3)


    return nc


_NC_CACHE = None
LAST_RESULT = None


def kernel(query, key, value, mask, Wq, bq, Wk, bk, Wv, bv):
    global _NC_CACHE, LAST_RESULT
    from concourse.bass_utils import run_bass_kernel_spmd

    bf16 = ml_dtypes.bfloat16
    query = np.asarray(query, np.float32)
    key = np.asarray(key, np.float32)
    value = np.asarray(value, np.float32)
    maskf = np.asarray(mask).astype(np.float32)
    Wqb = np.asarray(Wq, np.float32).astype(bf16)
    Wkb = np.asarray(Wk, np.float32).astype(bf16)
    Wvb = np.asarray(Wv, np.float32).astype(bf16)
    bq = np.asarray(bq, np.float32).reshape(D, 1)
    bk = np.asarray(bk, np.float32).reshape(D, 1)
    bv = np.asarray(bv, np.float32).reshape(D, 1)

    in_maps = []
    for c in range(NCORES):
        b, h = divmod(c, 2)
        qs = slice(h * SQ, (h + 1) * SQ)
        in_maps.append(
            {
                "qT": np.ascontiguousarray(query[b, qs].T).astype(bf16),
                "kT": np.ascontiguousarray(key[b].T).astype(bf16),
                "vT": np.ascontiguousarray(value[b].T).astype(bf16),
                "maskT": np.ascontiguousarray(maskf[b].reshape(TK, 128).T),
                "wq": Wqb,
                "wk": Wkb,
                "wv": Wvb,
                "bq": bq,
                "bk": bk,
                "bv": bv,
            }
        )

    if _NC_CACHE is None:
        _NC_CACHE = _build()

    res = run_bass_kernel_spmd(
        _NC_CACHE, in_maps, core_ids=list(range(NCORES))
    )
    LAST_RESULT = res

    outv = np.empty((B, S, D), np.float32)
    for c in range(NCORES):
        b, h = divmod(c, 2)
        outv[b, h * SQ : (h + 1) * SQ] = res.results[c]["outT"].T
    return outv
